# revision 19
# baseline (speedup 1.0000x reference)
"""Trainium2 Bass kernel v3 for 2-layer LSTM (B=64, T=512, D=64, H=512, out=32).

Data-parallel over batch (BL=8/core), weights replicated. v3 redesign vs v2:
- Gate tile order m = q*4 + g (q = output hidden chunk, g in [i,f,g,o]) so
  each hidden chunk q owns a contiguous 32-col block of the [128,128] PSUM
  gate tile; the sigmoid->c->tanh->h chain runs per-q, staggered so chains
  hide under the PE matmul phase of the same/next step (tail latency off the
  critical path; ~0.8us/step saved vs v2).
- Per-engine emission order matched to steady-state data arrival so the
  in-order Act/DVE queues don't head-of-line block across the 4 chains.
- h state ring [128, 2, 64] (16-col stride per chunk); bf16. An fp8 ring with
  DoubleRow matmuls (32 pairs/step) was tried and is still available via
  LSTM_HFP8/LSTM_DR env flags, but fp8 as the matmul MOVING operand degrades
  accuracy on this HW (~2e-2 vs 6e-3) regardless of scaling, so the default
  keeps fp8e3m4 weights on the stationary side only, bf16 h on the rhs.
- Layer-0 h mirrored to h1all (bf16, Pool engine, off-chain) for the bulk
  xg1 precompute; xg for both layers precomputed in bulk N=512 matmuls.
"""

import numpy as np
import ml_dtypes

import concourse.bass as bass
import concourse.mybir as mybir
import concourse.tile as tile
from concourse.bass_utils import run_bass_kernel_spmd

# ---------------------------------------------------------------------------
# walrus workaround: split the final TileContext drain (multi-sem-wait CTRL
# instruction) into one drain per proc; installed walrus caps waits at 1.
from concourse.vector_clock import ScopedClock, VectorClock


def _drain_and_barrier_split(self, tick_clock, wait_clock):
    gc = tick_clock.global_clock
    n = len(gc)
    emitted = 0
    for p in range(n):
        if gc[p] > 0:
            v = [0] * n
            v[p] = gc[p]
            d = self.nc.sync.drain()
            wait_clock.add_sem_waits(d.ins, ScopedClock({None: VectorClock(v)}))
            emitted += 1
    if emitted == 0:
        self.nc.sync.drain()
    self.nc.all_engine_barrier()
    assert self.sems is not None
    popped = self.nc._tile_sem_poison_stack.pop()
    assert popped is self._sem_poison
    self.nc.clear_and_free_semaphores(list(self.sems.allocated().values()))
    self.nc.all_engine_barrier()


tile.TileContext._drain_and_barrier = _drain_and_barrier_split

import bass_rust

_wsplit_ctr = [0]


def _split_multi_waits(nc):
    """walrus also caps waits at 1 on regular instructions: move extra waits
    onto same-engine NoOps inserted immediately before."""
    for fn in nc.m.functions:
        for blk in fn.blocks:
            insts = blk.instructions
            i = 0
            while i < len(insts):
                inst = insts[i]
                si = inst.sync_info
                if si is not None and len(si.on_wait) > 1:
                    waits = list(si.on_wait)
                    si.on_wait = [waits[-1]]
                    for w in waits[:-1]:
                        _wsplit_ctr[0] += 1
                        no = mybir.InstNoOp(
                            name=f"wsplit_{_wsplit_ctr[0]}", ins=[], outs=[])
                        no.engine = inst.engine
                        no.sync_info = bass_rust.SyncInfo(
                            on_wait=[w], on_update=[])
                        insts.insert(i, no)
                        i += 1
                i += 1
# ---------------------------------------------------------------------------

F32 = mybir.dt.float32
BF16 = mybir.dt.bfloat16
FP8E4 = mybir.dt.float8e4
FP8E3 = mybir.dt.float8e3
AF = mybir.ActivationFunctionType
ALU = mybir.AluOpType
DR = mybir.MatmulPerfMode.DoubleRow

import os
# fp8 as the matmul MOVING operand (required by DoubleRow) measurably degrades
# precision on this HW (rel err ~2e-2 vs 6e-3 with a bf16 rhs), so the h ring
# stays bf16 and W_hh stays fp8e3m4 on the stationary side only.
H_BF16 = os.environ.get("LSTM_HFP8", "") == ""
USE_DR = os.environ.get("LSTM_DR", "") != "" and not H_BF16
FS = 128.0          # weight scale; g gate gets 2*FS (tanh(x)=2*sig(2x)-1)
HS = 1.0 if H_BF16 else 16.0  # h-ring scale: keeps fp8e4m3 h out of the
                    # subnormal range (HW flushes fp8 subnormals); W_hh at FS/HS
HCOMP = 1.0
H_ON_POOL = False

B, D_IN, H, D_OUT = 64, 64, 512, 32
G = 4 * H          # 2048 gate rows; m-tile m = q*4 + g, rows (m%4)*512+(m//4)*128+p
BL = 8             # batch per core
NCORES = 8
KT = H // 128      # 4 hidden chunks
MT = G // 128      # 16 m-tiles


def _rec_layer(nc, pools, T, wdr, xg_dram, ring, c_sb, ident, h1all):
    """One LSTM layer recurrence, T steps, DoubleRow fp8e4.

    wdr: 2 SBUF tiles [128, MT*256] fp8e4 (kh-halves of W_hh, DR-packed)
    xg_dram: [T, 128, 128] bf16, cols = m*8 + b (FS-scaled xg+bias)
    ring: SBUF tile [128, 2, 64] fp8e4, cols q*16+b; slot t%2 holds h_t
    c_sb: [128, 32] f32 persistent cell state (cols q*8+b)
    h1all: optional [128, T, 32] bf16 tile to mirror h (layer 0), else None
    """
    xg_pool, sa_pool, x_pool, tn_pool, psum = pools
    for t in range(T):
        xg_t = xg_pool.tile([128, 128], BF16, name="xg", tag="xg")
        nc.sync.dma_start(xg_t[:], xg_dram[t])
        P = psum.tile([128, 128], F32, name="P", tag="P")
        # xg fold FIRST over all 128 cols (start=True zeroes + writes xg;
        # only depends on the DMA), then W_hh DR matmuls accumulate onto it.
        nc.tensor.matmul(P[:], ident[:], xg_t[:],
                         start=True, stop=False, skip_group_check=True)
        hs = ring[:, (t - 1) % 2, :]
        if USE_DR:
            for kh in (0, 1):
                rhs = hs[:, kh * 32:(kh + 1) * 32].rearrange(
                    "p (i b) -> p i b", i=2)[:, :, 0:BL]
                for m in range(MT):
                    lhsT = wdr[kh][:, m * 256:(m + 1) * 256].rearrange(
                        "p (i c) -> p i c", i=2)
                    nc.tensor.matmul(
                        P[:, m * 8:(m + 1) * 8], lhsT, rhs,
                        start=False, stop=(kh == 1), perf_mode=DR,
                        skip_group_check=True)
        else:
            for kk in range(4):
                rhs = hs[:, kk * 16:kk * 16 + BL]
                for m in range(MT):
                    lhsT = wdr[kk // 2][:, (kk % 2) * G + m * 128:
                                        (kk % 2) * G + (m + 1) * 128]
                    nc.tensor.matmul(
                        P[:, m * 8:(m + 1) * 8], lhsT, rhs,
                        start=False, stop=(kk == 3),
                        skip_group_check=True)

        sa = sa_pool.tile([128, 128], F32, name="sa", tag="sa")
        X = [None] * 4
        tn = [None] * 4

        def sig(q):
            nc.scalar.activation(sa[:, q * 32:(q + 1) * 32],
                                 P[:, q * 32:(q + 1) * 32],
                                 AF.Sigmoid, scale=1.0 / FS)

        def xop(q):
            X[q] = x_pool.tile([128, BL], F32, name="X", tag="X")
            # X = (sig_g - 0.5) * sig_i  == i*tanh(g_pre)/2
            nc.vector.scalar_tensor_tensor(
                X[q][:], sa[:, q * 32 + 16:q * 32 + 24], 0.5,
                sa[:, q * 32:q * 32 + 8],
                op0=ALU.subtract, op1=ALU.mult)

        def cfop(q):
            nc.vector.tensor_mul(c_sb[:, q * 8:(q + 1) * 8],
                                 c_sb[:, q * 8:(q + 1) * 8],
                                 sa[:, q * 32 + 8:q * 32 + 16])

        def cop(q):
            nc.vector.scalar_tensor_tensor(
                c_sb[:, q * 8:(q + 1) * 8], X[q][:], 2.0,
                c_sb[:, q * 8:(q + 1) * 8], op0=ALU.mult, op1=ALU.add)

        def tanh(q):
            tn[q] = tn_pool.tile([128, BL], F32, name="tn", tag="tn")
            nc.scalar.activation(tn[q][:], c_sb[:, q * 8:(q + 1) * 8], AF.Tanh)

        def hop(q):
            # ring holds HS*h = HS*sig_o*tanh(c)
            eng = nc.gpsimd if H_ON_POOL else nc.vector
            eng.scalar_tensor_tensor(
                ring[:, t % 2, q * 16:q * 16 + BL],
                sa[:, q * 32 + 24:q * 32 + 32], HS, tn[q][:],
                op0=ALU.mult, op1=ALU.mult)

        def hcopy(q):
            if h1all is not None:
                nc.gpsimd.tensor_mul(h1all[:, t, q * 8:(q + 1) * 8],
                                     sa[:, q * 32 + 24:q * 32 + 32], tn[q][:])

        # Emission order per engine matched to steady-state data arrival.
        # Act: sig0 sig1 sig2 tanh0 sig3 tanh1 tanh2 tanh3
        # DVE: X0 cf0 c0 X1 cf1 c1 X2 cf2 h0 c2 X3 cf3 h1 c3 h2 h3
        sig(0); sig(1)
        xop(0); cfop(0); cop(0)
        sig(2)
        xop(1); cfop(1); cop(1)
        tanh(0)
        sig(3)
        xop(2); cfop(2)
        tanh(1)
        hop(0); cop(2)
        xop(3); cfop(3)
        tanh(2)
        hop(1); cop(3)
        tanh(3)
        hop(2); hop(3)
        hcopy(0); hcopy(1); hcopy(2); hcopy(3)


def build_kernel(T):
    nc = bass.Bass()

    xT_d = nc.declare_dram_parameter("xT", [D_IN, T * BL], BF16, isOutput=False)
    wih0_d = nc.declare_dram_parameter("Wih0T", [D_IN, G], BF16, isOutput=False)
    WDT = FP8E4 if USE_DR else FP8E3
    whh0_d = nc.declare_dram_parameter("Whh0dr", [128, 2 * MT * 256], WDT,
                                       isOutput=False)
    wih1_d = nc.declare_dram_parameter("Wih1T", [H, G], BF16, isOutput=False)
    whh1_d = nc.declare_dram_parameter("Whh1dr", [128, 2 * MT * 256], WDT,
                                       isOutput=False)
    wout_d = nc.declare_dram_parameter("WoutT", [H, D_OUT], BF16, isOutput=False)
    b0_d = nc.declare_dram_parameter("b0", [1, G], BF16, isOutput=False)
    b1_d = nc.declare_dram_parameter("b1", [1, G], BF16, isOutput=False)
    bout_d = nc.declare_dram_parameter("bout", [D_OUT, 1], F32, isOutput=False)
    ident_d = nc.declare_dram_parameter("ident", [128, 128], BF16, isOutput=False)
    y_d = nc.declare_dram_parameter("yT", [D_OUT, BL], F32, isOutput=True)
    DEBUG = os.environ.get("LSTM_DEBUG", "") != ""
    if DEBUG:
        h1_d = nc.declare_dram_parameter("h1dbg", [128, T, 32], BF16,
                                         isOutput=True)
        xg_d = nc.declare_dram_parameter("xgdbg", [T, 128, 128], BF16,
                                         isOutput=True)

    xg0_d = nc.dram_tensor("xg0", [T, 128, 128], BF16)
    xg1_d = nc.dram_tensor("xg1", [T, 128, 128], BF16)

    NTOK = T * BL              # tokens per core
    NC_CHUNK = min(512, NTOK)  # precompute free-dim chunk
    n_chunks = NTOK // NC_CHUNK

    with tile.TileContext(nc) as tc:
        with (
            tc.tile_pool(name="w", bufs=1) as wpool,
            tc.tile_pool(name="xg", bufs=6) as xg_pool,
            tc.tile_pool(name="sa", bufs=3) as sa_pool,
            tc.tile_pool(name="xo", bufs=8) as x_pool,
            tc.tile_pool(name="tn", bufs=8) as tn_pool,
            tc.tile_pool(name="xgs", bufs=3) as xgs_pool,
            tc.tile_pool(name="psum_rec", bufs=4, space="PSUM") as psum_rec,
            tc.tile_pool(name="psum_pre", bufs=3, space="PSUM") as psum_pre,
        ):
            # ---- load weights / persistent state ----
            xT = wpool.tile([D_IN, NTOK], BF16, name="xT", tag="xT")
            nc.sync.dma_start(xT[:], xT_d[:])
            wih0 = wpool.tile([D_IN, G], BF16, name="wih0", tag="wih0")
            nc.sync.dma_start(wih0[:], wih0_d[:])
            wdr0 = [wpool.tile([128, MT * 256], WDT, name=f"wdr0_{k}",
                               tag=f"wdr0_{k}") for k in range(2)]
            wdr1 = [wpool.tile([128, MT * 256], WDT, name=f"wdr1_{k}",
                               tag=f"wdr1_{k}") for k in range(2)]
            wih1 = [wpool.tile([128, G], BF16, name=f"wih1_{k}", tag=f"wih1_{k}")
                    for k in range(KT)]
            wout = [wpool.tile([128, D_OUT], BF16, name=f"wout_{k}",
                               tag=f"wout_{k}") for k in range(KT)]
            for kh in range(2):
                sl = slice(kh * MT * 256, (kh + 1) * MT * 256)
                nc.sync.dma_start(wdr0[kh][:], whh0_d[:, sl])
                nc.sync.dma_start(wdr1[kh][:], whh1_d[:, sl])
            for k in range(KT):
                sl = slice(128 * k, 128 * (k + 1))
                nc.sync.dma_start(wih1[k][:], wih1_d[sl, :])
                nc.sync.dma_start(wout[k][:], wout_d[sl, :])
            b0 = wpool.tile([1, G], BF16, name="b0", tag="b0")
            nc.sync.dma_start(b0[:], b0_d[:])
            b1 = wpool.tile([1, G], BF16, name="b1", tag="b1")
            nc.sync.dma_start(b1[:], b1_d[:])
            bout = wpool.tile([D_OUT, 1], F32, name="bout", tag="bout")
            nc.sync.dma_start(bout[:], bout_d[:])
            ident = wpool.tile([128, 128], BF16, name="ident", tag="ident")
            nc.sync.dma_start(ident[:], ident_d[:])

            ones = wpool.tile([1, NC_CHUNK], BF16, name="ones", tag="ones")
            nc.gpsimd.memset(ones[:], 1.0)

            RING_DT = BF16 if H_BF16 else FP8E4
            h1all = wpool.tile([128, T, 32], BF16, name="h1all", tag="h1all")
            ring0 = wpool.tile([128, 2, 64], RING_DT, name="ring0", tag="ring0")
            nc.gpsimd.memset(ring0[:], 0.0)
            ring1 = wpool.tile([128, 2, 64], RING_DT, name="ring1", tag="ring1")
            nc.gpsimd.memset(ring1[:], 0.0)
            c0 = wpool.tile([128, 32], F32, name="c0", tag="c0")
            nc.gpsimd.memset(c0[:], 0.0)
            c1 = wpool.tile([128, 32], F32, name="c1", tag="c1")
            nc.gpsimd.memset(c1[:], 0.0)

            # ---- phase B/D: xg = W_ih @ src + b  (to DRAM, step-major) ----
            def emit_xg_precompute(lhs_tiles, rhs_src, bias, out_dram):
                for m in range(MT):
                    msl = slice(m * 128, (m + 1) * 128)
                    for c in range(n_chunks):
                        ps = psum_pre.tile([128, NC_CHUNK], F32, name="pre",
                                           tag="pre")
                        nkk = len(lhs_tiles)
                        for kk in range(nkk):
                            nc.tensor.matmul(
                                ps[:], lhs_tiles[kk][:, msl],
                                rhs_src(kk, c),
                                start=(kk == 0), stop=False)
                        nc.tensor.matmul(
                            ps[:], bias[0:1, msl], ones[0:1, :],
                            start=False, stop=True)
                        xgsb = xgs_pool.tile(
                            [128, NC_CHUNK], BF16, name="xgsb", tag="xgsb")
                        nc.vector.tensor_copy(xgsb[:], ps[:])
                        t0 = c * (NC_CHUNK // BL)
                        nt = NC_CHUNK // BL
                        nc.sync.dma_start(
                            out_dram[t0:t0 + nt, :, m * 8:(m + 1) * 8]
                            .rearrange("t p b -> p t b"),
                            xgsb[:].rearrange("p (t b) -> p t b", b=BL),
                        )

            emit_xg_precompute(
                [wih0],
                lambda kk, c: xT[:, c * NC_CHUNK:(c + 1) * NC_CHUNK],
                b0, xg0_d)

            # ---- phase C: layer-0 recurrence ----
            pools = (xg_pool, sa_pool, x_pool, tn_pool, psum_rec)
            _rec_layer(nc, pools, T, wdr0, xg0_d, ring0, c0, ident, h1all)

            if DEBUG:
                nc.sync.dma_start(h1_d[:], h1all[:])
                for tdb in range(T):
                    dbg = xgs_pool.tile([128, 128], BF16, name="xgsb", tag="xgsb")
                    nc.sync.dma_start(dbg[:], xg0_d[tdb])
                    nc.sync.dma_start(xg_d[tdb], dbg[:])

            # ---- phase D: xg1 = W_ih1 @ h1.T + b1 ----
            TB = NC_CHUNK // BL  # steps per chunk
            emit_xg_precompute(
                wih1,
                lambda kk, c: h1all[:, c * TB:(c + 1) * TB, kk * 8:kk * 8 + 8],
                b1, xg1_d)

            # ---- phase E: layer-1 recurrence ----
            _rec_layer(nc, pools, T, wdr1, xg1_d, ring1, c1, ident, None)

            # ---- phase F: y.T = W_out @ h_last.T + b_out ----
            ps_y = psum_rec.tile([D_OUT, BL], F32, name="P", tag="P")
            last = (T - 1) % 2
            for kk in range(KT):
                nc.tensor.matmul(
                    ps_y[:], wout[kk][:],
                    ring1[:, last, kk * 16:kk * 16 + BL],
                    start=(kk == 0), stop=(kk == KT - 1),
                )
            y_sb = sa_pool.tile([D_OUT, BL], F32, name="y_sb", tag="y_sb")
            nc.scalar.activation(y_sb[:], ps_y[:], AF.Identity, bias=bout[:, 0:1])
            nc.sync.dma_start(y_d[:], y_sb[:])

    _split_multi_waits(nc)
    return nc


_NC_CACHE = {}


def _get_nc(T):
    if T not in _NC_CACHE:
        _NC_CACHE[T] = build_kernel(T)
    return _NC_CACHE[T]


def _gscale(W):
    """W: [4H, ...] gate-major rows [i,f,g,o]: scale by FS, 2*FS on g gate."""
    W = np.array(W, dtype=np.float32, copy=True)
    W *= FS
    W[2 * H:3 * H] *= 2.0
    return W


def _perm_mtile(W):
    """Permute gate-major rows [4H, X] into m-tile order m=q*4+g:
    out[m*128+p] = W[(m%4)*512 + (m//4)*128 + p]."""
    X = W.reshape(4, 4, 128, -1)       # [g, q, p, cols]
    return X.transpose(1, 0, 2, 3).reshape(G, -1)  # [q, g, p, cols]


def _whh_dr_pack(W):
    """W_hh [2048, 512] -> [128, 2*MT*256] fp8e4m3.
    DR mode: host[k, kh*4096 + m*256 + i*128 + p] = FSW[r(m,p), (2kh+i)*128+k]
    non-DR:  host[k, kk*2048 + m*128 + p] = FSW[r(m,p), kk*128+k]."""
    FSW = _gscale(W) * (HCOMP / HS)        # [2048, 512], net scale FS/HS
    W5 = FSW.reshape(4, 4, 128, 4, 128)    # [g, q, p, kc, k]
    if USE_DR:
        A = W5.transpose(4, 3, 1, 0, 2)        # [k, kc, q, g, p]
        A = A.reshape(128, 2, 2, 4, 4, 128)    # [k, kh, i, q, g, p]
        A = A.transpose(0, 1, 3, 4, 2, 5)      # [k, kh, q, g, i, p]
    else:
        A = W5.transpose(4, 3, 1, 0, 2)        # [k, kc, q, g, p]
    wdt = ml_dtypes.float8_e4m3 if USE_DR else ml_dtypes.float8_e3m4
    return np.ascontiguousarray(
        A.reshape(128, 2 * MT * 256)).astype(wdt)


def _prep_inputs(x, W_ih0, W_hh0, b_ih0, b_hh0, W_ih1, W_hh1, b_ih1, b_hh1,
                 W_out, b_out):
    bf = ml_dtypes.bfloat16
    T = x.shape[1]
    # wih: [D, G] cols in m-tile order
    wih0 = np.ascontiguousarray(_perm_mtile(_gscale(W_ih0)).T)    # [64, 2048]
    wih1 = np.ascontiguousarray(_perm_mtile(_gscale(W_ih1)).T)    # [512, 2048]
    b0 = _perm_mtile(_gscale((b_ih0 + b_hh0).reshape(G, 1))).reshape(1, G)
    b1 = _perm_mtile(_gscale((b_ih1 + b_hh1).reshape(G, 1))).reshape(1, G)
    shared = {
        "Wih0T": wih0.astype(bf),
        "Whh0dr": _whh_dr_pack(W_hh0),
        "Wih1T": wih1.astype(bf),
        "Whh1dr": _whh_dr_pack(W_hh1),
        "WoutT": np.ascontiguousarray(W_out.T * (HCOMP / HS)).astype(bf),
        "b0": b0.astype(bf),
        "b1": b1.astype(bf),
        "bout": b_out.reshape(D_OUT, 1).astype(np.float32),
        "ident": np.eye(128, dtype=np.float32).astype(bf),
    }
    in_maps = []
    for c in range(NCORES):
        xc = x[c * BL:(c + 1) * BL]            # [8, T, 64]
        xT = np.ascontiguousarray(xc.transpose(2, 1, 0).reshape(D_IN, T * BL))
        in_maps.append({"xT": xT.astype(bf), **shared})
    return in_maps


def kernel(x, W_ih0, W_hh0, b_ih0, b_hh0, W_ih1, W_hh1, b_ih1, b_hh1,
           W_out, b_out):
    T = x.shape[1]
    nc = _get_nc(T)
    in_maps = _prep_inputs(x, W_ih0, W_hh0, b_ih0, b_hh0, W_ih1, W_hh1,
                           b_ih1, b_hh1, W_out, b_out)
    res = run_bass_kernel_spmd(nc, in_maps, core_ids=list(range(NCORES)))
    out = np.concatenate(
        [res.results[c]["yT"].T for c in range(NCORES)], axis=0)
    return np.ascontiguousarray(out.astype(np.float32))
